# revision 20
# baseline (speedup 1.0000x reference)
"""GCN layer kernel for Trainium2, 8-core row-parallel.

Computes out = (adj * mask + I) @ (x @ W^T) for N=8192, C_in=C_out=128.

Sharding: adj/mask row-blocks of 1024 across 8 cores; x, W replicated.
Final design (SWDGE bf16-cast stream; 192-196us HW exec vs the 228us
v1 baseline, rel err 3.1e-3):
  - each core's adj/mask row-slice is uploaded TRANSPOSED (adjT/maskT =
    [N, R] f32, a pure host-side layout choice).  The DMA lands A with
    k on partitions natively, deleting the entire device-side transpose
    pipeline of v1/v2.  PE per 4MB chunk is just 12 instructions.
  - the adj/mask stream rides the gpsimd SWDGE queue with inline
    f32->bf16 cast (bf16 SBUF tiles): muls are all-bf16 (2 elem/cyc on
    DVE), SBUF tiles halve so pools go 6 deep, and DVE can never gate
    the stream -- the post-stream tail is one mul + matmuls + finalize.
  - 4KB f32 source descriptors (1024 elements, within the SDMA cast
    path's per-descriptor element limit; 16KB descriptors regressed
    35us in v6).  The stream sustains ~422 GB/s of HBM reads, ~97% of
    the per-core 16-SDMA-engine aggregate (~435 GB/s), which is the
    binding resource for this kernel.  Do NOT reorder the W load
    behind stream triggers in the Q7 queue (v7 regressed 34us: W
    lands late, phase-0 stalls, prod-slot recycling cascades).
  - xT = x.T ([C, N] f32) loads on the otherwise-idle SP HWDGE ring
    (32KB/partition contiguous descriptors, 2 parts), so the main
    stream starts at t~1us; phase-0 h-tiles are one f32 matmul each
    (stationary = xT column block), hoisted early by the scheduler.
  - the k-axis is rotated by r0 per core (host-side, pure accumulation
    order change) so each core's self-loop x block sits at a fixed xT
    column range for the SPMD-shared program.
  - chunk q covers k in [512q, 512q+512) and ALL 1024 output rows; per
    k-128-block b one stationary h-tile serves BOTH output blocks.
  - accumulation f32 in PSUM; rel err ~4e-3 vs the 2e-2 gate (adj and
    mask are bf16-rounded before the product).
  - last chunk streams in 4 quarter-DMAs with per-quarter muls;
    finalize transposes pacc back to row-major via PE, adds the
    self-loop h, writes out with 2KB descriptors on the SP ring.
"""

import numpy as np
from contextlib import ExitStack

from concourse import bass, bacc, tile, mybir
from concourse import masks
from concourse.bass_utils import run_bass_kernel_spmd

N = 8192
C = 128
NCORES = 8
R = N // NCORES          # 1024 rows per core
M_BLK = 512              # psum accumulation block (free dim of main matmul)
NBLK = R // M_BLK        # 2 m-blocks per core
KB = 512                 # k-width per chunk
B = KB // 128            # 4 k-128-blocks per chunk
NCH = N // KB            # 16 chunks
JF = 4                   # finalize: rows per partition (out descriptor = JF*512B)
PREF = 4                 # chunks of DMA-trigger prefetch ahead of compute

F32 = mybir.dt.float32
BF16 = mybir.dt.bfloat16


def build_program():
    nc = bacc.Bacc("TRN2", target_bir_lowering=False, debug=False, num_devices=NCORES)

    adjT_d = nc.dram_tensor("adjT", [N, R], F32, kind="ExternalInput").ap()
    maskT_d = nc.dram_tensor("maskT", [N, R], F32, kind="ExternalInput").ap()
    xT_d = nc.dram_tensor("xT", [C, N], F32, kind="ExternalInput").ap()
    w_d = nc.dram_tensor("w", [C, C], F32, kind="ExternalInput").ap()
    out_d = nc.dram_tensor("out", [R, C], F32, kind="ExternalOutput").ap()

    with tile.TileContext(nc) as tc, ExitStack() as ctx:
        const_pool = ctx.enter_context(tc.tile_pool(name="const", bufs=1))
        h_pool = ctx.enter_context(tc.tile_pool(name="h", bufs=1))
        adj_pool = ctx.enter_context(tc.tile_pool(name="adj", bufs=6))
        mask_pool = ctx.enter_context(tc.tile_pool(name="mask", bufs=6))
        prod_pool = ctx.enter_context(tc.tile_pool(name="prod", bufs=4))
        fin_pool = ctx.enter_context(tc.tile_pool(name="fin", bufs=4))
        psum_acc = ctx.enter_context(tc.tile_pool(name="pacc", bufs=2, space="PSUM"))
        psum_misc = ctx.enter_context(tc.tile_pool(name="pmisc", bufs=3, space="PSUM"))
        psum_fin = ctx.enter_context(tc.tile_pool(name="pfin", bufs=1, space="PSUM"))

        # ---- xT on the SP HWDGE ring: 2 parts, 16KB/partition each ----
        xTf = const_pool.tile([128, N], F32)
        for p in range(2):
            csl = slice(p * (N // 2), (p + 1) * (N // 2))
            nc.sync.dma_start(out=xTf[:, csl], in_=xT_d[:, csl])

        ident = const_pool.tile([128, 128], F32)
        masks.make_identity(nc, ident[:])

        # ---- weight: W^T in f32 (phase-0 matmuls are all-f32) ----
        w_sb = const_pool.tile([128, C], F32)
        nc.gpsimd.dma_start(out=w_sb[:], in_=w_d[:, :])
        psum_wt = psum_misc.tile([128, 128], F32, tag="pm")
        nc.tensor.transpose(psum_wt[:], w_sb[:], ident[:])
        wtr_sb = const_pool.tile([128, C], F32)
        nc.vector.tensor_copy(wtr_sb[:], psum_wt[:])

        # h tile kg holds rows kg*128 + p (natural order), bf16
        h_sb = h_pool.tile([128, N // 128, C], BF16)
        ho_sb = const_pool.tile([128, NBLK * JF, C], BF16)

        def h_tile_pipe(stat_view, dst_view):
            # h-block = (xT cols)^T @ W^T; stationary = xT column block
            psum_h = psum_misc.tile([128, 128], F32, tag="pm")
            nc.tensor.matmul(psum_h[:], stat_view, wtr_sb[:], start=True, stop=True)
            nc.scalar.copy(dst_view, psum_h[:])

        def phase0_group(g):
            # h tiles for x rows [g*1024, (g+1)*1024)
            for j in range(8):
                kg = g * 8 + j
                h_tile_pipe(
                    xTf[:, kg * 128 : (kg + 1) * 128], h_sb[:, kg, :]
                )

        def phase0_own():
            # self-loop h in the finalize permutation (row blk*512 + JF*p + j).
            # The k-axis is rotated per-core on the host so this core's own
            # rows are xT columns [0, R): a fixed range in the SPMD program.
            for blk in range(NBLK):
                v = xTf[:, blk * M_BLK : (blk + 1) * M_BLK].rearrange(
                    "p (m j) -> p j m", j=JF
                )
                for j in range(JF):
                    h_tile_pipe(v[:, j, :], ho_sb[:, blk * JF + j, :])

        # ---- main loop ----
        def emit_triggers(q, parts):
            k0 = q * KB
            adj_t = adj_pool.tile([128, B, R], BF16, tag="adj")
            mask_t = mask_pool.tile([128, B, R], BF16, tag="mask")
            bw = B // parts          # k-128-blocks per part
            for hh in range(parts):
                bsl = slice(hh * bw, (hh + 1) * bw)
                rsl = slice(k0 + hh * bw * 128, k0 + (hh + 1) * bw * 128)
                nc.gpsimd.dma_start(
                    out=adj_t[:, bsl, :],
                    in_=adjT_d[rsl, :].rearrange("(b p) m -> p b m", p=128),
                )
                nc.gpsimd.dma_start(
                    out=mask_t[:, bsl, :],
                    in_=maskT_d[rsl, :].rearrange("(b p) m -> p b m", p=128),
                )
            return adj_t, mask_t

        def emit_muls(adj_t, mask_t, parts=2):
            # separate bf16 product tile: adj AND mask slots free at the mul
            prod_t = prod_pool.tile([128, B, R], BF16, tag="prod")
            bw = B // parts
            for hh in range(parts):
                bsl = slice(hh * bw, (hh + 1) * bw)
                nc.vector.tensor_mul(
                    prod_t[:, bsl, :], adj_t[:, bsl, :], mask_t[:, bsl, :]
                )
            return prod_t

        def emit_kbmms(q, prod_t, paccs):
            for b in range(B):
                kg = q * B + b
                st = kg == 0
                sp = kg == N // 128 - 1
                for blk in range(NBLK):
                    nc.tensor.matmul(
                        paccs[blk][:],
                        h_sb[:, kg, :],
                        prod_t[:, b, blk * M_BLK : (blk + 1) * M_BLK],
                        start=st,
                        stop=sp,
                    )

        def finalize(blk, pacc):
            # out rows blk*512 + JF*p + j; 2KB out descriptors
            psum_nat = psum_fin.tile([128, JF, C], F32)
            pacc_j = pacc[:].rearrange("p (m j) -> p j m", j=JF)
            for j in range(JF):
                otj = fin_pool.tile([128, 128], F32, tag="fin_t")
                nc.vector.tensor_copy(otj[:], pacc_j[:, j, :])
                nc.tensor.transpose(psum_nat[:, j, :], otj[:], ident[:])
            out_sb = fin_pool.tile([128, JF, C], F32, tag="fin_o")
            nc.vector.tensor_add(
                out_sb[:],
                psum_nat[:],
                ho_sb[:, blk * JF : (blk + 1) * JF, :],
            )
            r0 = blk * M_BLK
            nc.sync.dma_start(
                out=out_d[r0 : r0 + M_BLK, :].rearrange("(p j) c -> p j c", p=128),
                in_=out_sb[:],
            )

        def trig_parts(q):
            # halves everywhere (fine-grained mul deps); last chunk in
            # quarters so the final muls/matmuls start as early as possible
            return 4 if q == NCH - 1 else 2

        last = NCH - 1
        paccs = {
            blk: psum_acc.tile([128, M_BLK], F32, name="pacc")
            for blk in range(NBLK)
        }
        trigs = {}
        for k in range(PREF):
            trigs[k] = emit_triggers(k, parts=2)
        for q in range(NCH):
            if q + PREF <= last:
                trigs[q + PREF] = emit_triggers(
                    q + PREF, parts=trig_parts(q + PREF)
                )
            prod_t = emit_muls(*trigs.pop(q), parts=trig_parts(q))
            if q % 2 == 0:
                phase0_group(q // 2)
            if q == 8:
                phase0_own()
            emit_kbmms(q, prod_t, paccs)
        finalize(0, paccs[0])
        finalize(1, paccs[1])

    nc.compile()
    return nc


_NC_CACHE = None


def _get_nc():
    global _NC_CACHE
    if _NC_CACHE is None:
        _NC_CACHE = build_program()
    return _NC_CACHE


def _rolled_T(a, r0):
    # ascontiguousarray(np.roll(a, -r0, axis=1).T) in one transpose-copy:
    # row k' of the result is column (r0 + k') % N of `a`
    n = a.shape[1]
    out = np.empty((n, a.shape[0]), dtype=np.float32)
    out[: n - r0] = a[:, r0:].T
    out[n - r0 :] = a[:, :r0].T
    return out


def make_in_maps(x, adj, mask, W):
    W = np.ascontiguousarray(W, dtype=np.float32)
    in_maps = []
    for i in range(NCORES):
        r0 = i * R
        # the k-axis is rotated by r0 per core (pure accumulation-order
        # change) so each core's own rows sit at k' in [0, R) -- the
        # SPMD-shared program can then address the self-loop x block at
        # a fixed location.  adjT/maskT/xT all use the same rotation.
        in_maps.append(
            {
                "adjT": _rolled_T(adj[r0 : r0 + R], r0),
                "maskT": _rolled_T(mask[r0 : r0 + R], r0),
                "xT": np.ascontiguousarray(_rolled_T(x.T, r0).T),
                "w": W,
            }
        )
    return in_maps


def kernel(x, adj, mask, W):
    nc = _get_nc()
    in_maps = make_in_maps(x, adj, mask, W)
    res = run_bass_kernel_spmd(nc, in_maps, list(range(NCORES)))
    return np.concatenate([res.results[i]["out"] for i in range(NCORES)], axis=0)
